# revision 33
# baseline (speedup 1.0000x reference)
"""Adjacency-aware multi-head attention on 8 trn2 NeuronCores.

Math (per b, head k):
  Q = h[b] @ Wq[:, k] + bq[k]           [N, D]
  S[i, j] = (Q_i . K_j) / sqrt(D)
  P[j, i] = exp(S[i, j]) / sum_j exp(S[i, j])      (softmax over keys j)
  out[i, d] = sum_j P[j, i] * A[b, j, i] * V[j, d]

Sharding: 16 (b, head) pairs over 8 cores, 2 heads of the SAME b per core so
the A[b] stream is shared by both heads.

Device dataflow ([j, i] "transposed" layout so A needs no transpose):
  S^T[j-tile, i-chunk] on PE: K^T tile stationary (bf16), Q^T moving (bf16).
    K^T is packed so j-tile t lives on partition strip 32*(t%4) and Q^T is
    replicated to all 4 strips -> 4 j-tiles run CONCURRENTLY on the PE's
    four 32-row groups.
  exp on ACT: PSUM -> SBUF bf16 (groups of 3 PSUM banks per op)
  EA = E * A on DVE (bf16 tensor_tensor, 2x mode; A broadcast over heads)
  Phase 2 (lagged one chunk behind S so the in-order PE queue never stalls),
  4 concurrent col-group accumulation streams into one PSUM tile:
    cols  0-31: outT_h0 += V_h0[j]^T @ EA_h0     (M=32)
    cols 32-63: outT_h1 += V_h1[j]^T @ EA_h1     (M=32)
    col  64: denom_h0 += ones^T @ E_h0           (M=1)
    col  96: denom_h1 += ones^T @ E_h1           (M=1)
Device returns [128, N]: rows 0-31 outT_h0, 32-63 outT_h1, rows 64/96 the
softmax denominators.  Host does out = (outT / denom)^T plus the gather.
"""

import math
import os

import numpy as np
import ml_dtypes

B, N, IN_DIM = 2, 2048, 256
HEADS, D = 8, 32
NCORES = 8
HPC = 2              # heads per core
NJ = N // 128        # 16 j-tiles
NCH = 4              # i-chunks
CH = N // NCH        # 512
CORES_PER_B = NCORES // B

LAST_RESULTS = None  # BassKernelResults of the most recent kernel() call


def _build_bass():
    import concourse.bass as bass
    import concourse.mybir as mybir
    import concourse.tile as tile
    from concourse import bacc

    f32 = mybir.dt.float32
    bf16 = mybir.dt.bfloat16
    AF = mybir.ActivationFunctionType

    nc = bacc.Bacc("TRN2", target_bir_lowering=False, debug=False,
                   num_devices=NCORES)

    hT = nc.dram_tensor("hT", [IN_DIM, N], bf16, kind="ExternalInput").ap()
    Ab = nc.dram_tensor("Ab", [N, N], bf16, kind="ExternalInput").ap()
    wq = nc.dram_tensor("wq", [IN_DIM, HPC * D], bf16, kind="ExternalInput").ap()
    wk = nc.dram_tensor("wk", [IN_DIM, HPC * D], bf16, kind="ExternalInput").ap()
    wv = nc.dram_tensor("wv", [IN_DIM, HPC * D], bf16, kind="ExternalInput").ap()
    bq = nc.dram_tensor("bq", [HPC * D, 1], f32, kind="ExternalInput").ap()
    bk4 = nc.dram_tensor("bk4", [128, HPC], f32, kind="ExternalInput").ap()
    bvb = nc.dram_tensor("bvb", [128, HPC * D], f32, kind="ExternalInput").ap()
    o = nc.dram_tensor("o", [128, N], f32, kind="ExternalOutput").ap()

    SC = 1.0 / math.sqrt(D)

    with (
        tile.TileContext(nc) as tc,
        tc.tile_pool(name="const", bufs=1) as cpool,
        tc.tile_pool(name="ps", bufs=2, space="PSUM") as pspool,
        tc.tile_pool(name="pod", bufs=2, space="PSUM") as podpool,
        tc.tile_pool(name="apool", bufs=2) as apool,
        tc.tile_pool(name="epool", bufs=2) as epool,
        tc.tile_pool(name="eapool", bufs=2) as eapool,
        tc.tile_pool(name="opool", bufs=3) as opool,
    ):
        # ---- constants / inputs into SBUF
        hT_sb = cpool.tile([128, 2, N], bf16, tag="hT")
        nc.sync.dma_start(hT_sb, hT.rearrange("(s p) n -> p s n", p=128))
        w_sb = {}
        for name, ap in (("q", wq), ("k", wk), ("v", wv)):
            t = cpool.tile([128, 2, HPC * D], bf16, tag=f"w{name}")
            nc.sync.dma_start(t, ap.rearrange("(s p) m -> p s m", p=128))
            w_sb[name] = t
        bq_sb = cpool.tile([HPC * D, 1], f32, tag="bq")
        nc.sync.dma_start(bq_sb, bq)
        bk4_sb = cpool.tile([128, HPC], f32, tag="bk4")
        nc.sync.dma_start(bk4_sb, bk4)
        bvb_sb = cpool.tile([128, HPC * D], f32, tag="bvb")
        nc.sync.dma_start(bvb_sb, bvb)
        ones_sb = cpool.tile([128, 1], bf16, tag="ones")
        nc.vector.memset(ones_sb, 1.0)

        # ---- HAM warm-up: dense dummy matmuls during the input-DMA wait so
        #      the PE clock is at 2.4 GHz when real work starts
        warm_sb = cpool.tile([128, 640], bf16, tag="warm")
        nc.vector.memset(warm_sb, 0.0)
        for w in range(24):
            wps = podpool.tile([128, CH], f32, tag="od")
            nc.tensor.matmul(
                wps, lhsT=warm_sb[:, 0:128], rhs=warm_sb[:, 128:640],
                start=True, stop=True,
            )

        # Q^T replicated on all 4 partition strips; K^T packed so j-tile t
        # sits on strip 32*(t%4), column block t//4.
        qt4 = [cpool.tile([128, N], bf16, tag=f"qt4{h}", name=f"qt4{h}")
               for h in range(HPC)]
        kt4 = [cpool.tile([128, NJ // 4, 128], bf16, tag=f"kt4{h}",
                          name=f"kt4{h}") for h in range(HPC)]
        Vt = cpool.tile([128, NJ, HPC * D], bf16, tag="vt")

        def bcast_free(ap_col, n):
            return bass.AP(tensor=ap_col.tensor, offset=ap_col.offset,
                           ap=[ap_col.ap[0], [0, n]])

        # ---- Q projection: QT[64, N] (+bias, scaled 1/sqrt(D)), then
        #      strip-replicate into qt4 via SBUF->SBUF DMA
        bq_bcast = bcast_free(bq_sb, CH)
        for quarter in range(4):
            sl = slice(quarter * CH, (quarter + 1) * CH)
            ps = pspool.tile([HPC * D, CH], f32, tag="ps")
            for s in range(2):
                nc.tensor.matmul(
                    ps, lhsT=w_sb["q"][:, s, :], rhs=hT_sb[:, s, sl],
                    start=(s == 0), stop=(s == 1),
                )
            nc.vector.scalar_tensor_tensor(
                qt4[0][0:D, sl], ps[0:D, :], SC, bq_bcast[0:D, :],
                op0=mybir.AluOpType.mult, op1=mybir.AluOpType.add,
            )
            nc.vector.scalar_tensor_tensor(
                qt4[1][D:2 * D, sl], ps[D:2 * D, :], SC, bq_bcast[D:2 * D, :],
                op0=mybir.AluOpType.mult, op1=mybir.AluOpType.add,
            )
        for r in range(1, 4):
            nc.sync.dma_start(qt4[0][32 * r:32 * r + 32, :], qt4[0][0:D, :])
        for r in (0, 2, 3):
            nc.sync.dma_start(qt4[1][32 * r:32 * r + 32, :], qt4[1][D:2 * D, :])

        # ---- K projection straight into the packed kt4 layout: for strip r
        #      the moving operand picks j-tiles {r, r+4, r+8, r+12}
        for h in range(HPC):
            ps = pspool.tile([128, 4 * 128], f32, tag="ps")
            for r in range(4):
                for s in range(2):
                    base = hT_sb[:, s, r * 128:(r + 1) * 128]
                    rhs = bass.AP(
                        tensor=base.tensor, offset=base.offset,
                        ap=[base.ap[0], [4 * 128, 4], [1, 128]],
                    )
                    nc.tensor.matmul(
                        ps[32 * r:32 * r + 32, :],
                        lhsT=w_sb["k"][:, s, h * D:(h + 1) * D],
                        rhs=rhs,
                        start=(s == 0), stop=(s == 1),
                        tile_position=(0, 32 * r),
                    )
            nc.vector.scalar_tensor_tensor(
                kt4[h].rearrange("p q jj -> p (q jj)"), ps, 1.0,
                bcast_free(bk4_sb[:, h:h + 1], 4 * 128),
                op0=mybir.AluOpType.mult, op1=mybir.AluOpType.add,
            )

        # ---- projection V[j, d] for both heads (+bias via broadcast tile)
        for t in range(NJ):
            ps = pspool.tile([128, HPC * D], f32, tag="ps")
            for s in range(2):
                nc.tensor.matmul(
                    ps, lhsT=hT_sb[:, s, t * 128:(t + 1) * 128],
                    rhs=w_sb["v"][:, s, :],
                    start=(s == 0), stop=(s == 1),
                )
            nc.vector.tensor_add(Vt[:, t, :], ps, bvb_sb)

        # ---- main loop (phase2 lags one chunk so the in-order PE queue
        #      never stalls waiting on exp/A-mult of the same chunk)
        A3 = Ab.rearrange("(t p) i -> p t i", p=128)

        # e/ea column block for (head hh, j-tile t):
        def blk(hh, t):
            return (t // 4) * 8 + hh * 4 + (t % 4)

        def emit_phase2_tile(od, e_t, ea_t, t):
            first, last = (t == 0), (t == NJ - 1)
            sh0 = slice(blk(0, t) * CH, (blk(0, t) + 1) * CH)
            sh1 = slice(blk(1, t) * CH, (blk(1, t) + 1) * CH)
            nc.tensor.matmul(
                od[0:D, :], lhsT=Vt[:, t, 0:D], rhs=ea_t[:, sh0],
                start=first, stop=last, tile_position=(0, 0),
            )
            nc.tensor.matmul(
                od[D:2 * D, :], lhsT=Vt[:, t, D:2 * D], rhs=ea_t[:, sh1],
                start=first, stop=last, tile_position=(0, 32),
            )
            nc.tensor.matmul(
                od[64:65, :], lhsT=ones_sb, rhs=e_t[:, sh0],
                start=first, stop=last, tile_position=(0, 64),
            )
            nc.tensor.matmul(
                od[96:97, :], lhsT=ones_sb, rhs=e_t[:, sh1],
                start=first, stop=last, tile_position=(0, 96),
            )

        def emit_phase2(e_t, ea_t, ch):
            od = podpool.tile([128, CH], f32, tag="od")
            for t in range(NJ):
                emit_phase2_tile(od, e_t, ea_t, t)
            o_sb = opool.tile([128, CH], f32, tag="o")
            nc.vector.tensor_copy(o_sb, od)
            nc.sync.dma_start(o[:, ch * CH:(ch + 1) * CH], o_sb)

        GRP = 3              # S psum banks per exp op
        nblocks = NJ * HPC
        pending = None
        for ch in range(NCH):
            a_t = apool.tile([128, NJ * CH], bf16, tag="a")
            nc.sync.dma_start(
                a_t.rearrange("p (t i) -> p t i", i=CH),
                A3[:, :, ch * CH:(ch + 1) * CH],
            )
            e_t = epool.tile([128, nblocks * CH], bf16, tag="e")
            ea_t = eapool.tile([128, nblocks * CH], bf16, tag="ea")
            # S matmuls: block b = q*8 + hh*4 + r covers j-tile t = 4q + r;
            # runs of 4 same-head blocks occupy the 4 PE row-groups and run
            # concurrently.  exp drains GRP psum banks per op.  phase2 MMs
            # of the previous chunk are interleaved to fill exp-gated PE
            # stalls.
            ps = None
            gsz = 0
            for b in range(nblocks):
                q, hh, r = b // 8, (b // 4) % 2, b % 4
                g = b % GRP
                if g == 0:
                    gsz = min(GRP, nblocks - b)
                    ps = pspool.tile([128, GRP * CH], f32, tag="ps")
                nc.tensor.matmul(
                    ps[:, g * CH:(g + 1) * CH],
                    lhsT=kt4[hh][32 * r:32 * r + 32, q, :],
                    rhs=qt4[hh][32 * r:32 * r + 32, ch * CH:(ch + 1) * CH],
                    start=True, stop=True,
                    tile_position=(32 * r, 0),
                )
                if g == gsz - 1:
                    b0 = b - g
                    nc.scalar.activation(
                        e_t[:, b0 * CH:(b + 1) * CH], ps[:, :gsz * CH], AF.Exp)
            for t in range(NJ):
                b0 = blk(0, t)
                base = e_t[:, b0 * CH:(b0 + 1) * CH]
                e_pair = bass.AP(tensor=base.tensor, offset=base.offset,
                                 ap=[base.ap[0], [4 * CH, 2], [1, CH]])
                base2 = ea_t[:, b0 * CH:(b0 + 1) * CH]
                ea_pair = bass.AP(tensor=base2.tensor, offset=base2.offset,
                                  ap=[base2.ap[0], [4 * CH, 2], [1, CH]])
                a_sl = a_t[:, t * CH:(t + 1) * CH]
                a_bcast = bass.AP(tensor=a_sl.tensor, offset=a_sl.offset,
                                  ap=[a_sl.ap[0], [0, HPC], a_sl.ap[1]])
                nc.vector.tensor_mul(ea_pair, e_pair, a_bcast)
            if pending is not None:
                emit_phase2(*pending)
            pending = (e_t, ea_t, ch)
        emit_phase2(*pending)

    nc.finalize()
    return nc


def kernel(h, A, Wq, bq, Wk, bk, Wv, bv):
    global LAST_RESULTS
    from concourse.bass_utils import run_bass_kernel_spmd

    h = np.asarray(h, np.float32)
    A = np.asarray(A, np.float32)
    Wq = np.asarray(Wq, np.float32)
    Wk = np.asarray(Wk, np.float32)
    Wv = np.asarray(Wv, np.float32)
    bq = np.asarray(bq, np.float32)
    bk = np.asarray(bk, np.float32)
    bv = np.asarray(bv, np.float32)

    hT = np.ascontiguousarray(h.transpose(0, 2, 1)).astype(ml_dtypes.bfloat16)
    Ab = np.ascontiguousarray(A.astype(ml_dtypes.bfloat16))  # [B, N, N]
    sc = np.float32(1.0 / math.sqrt(D))

    in_maps = []
    for c in range(NCORES):
        b = c // CORES_PER_B
        h0 = HPC * (c % CORES_PER_B)
        sl = slice(h0 * D, (h0 + HPC) * D)
        bk2 = bk[sl].reshape(HPC, D)                    # [head, d]
        bk4 = np.empty((128, HPC), np.float32)
        for hh in range(HPC):
            bk4[:, hh] = np.tile(bk2[hh], 4)            # strip-replicated
        in_maps.append({
            "hT": hT[b],
            "Ab": Ab[b],
            "wq": np.ascontiguousarray(Wq[:, sl]).astype(ml_dtypes.bfloat16),
            "wk": np.ascontiguousarray(Wk[:, sl]).astype(ml_dtypes.bfloat16),
            "wv": np.ascontiguousarray(Wv[:, sl]).astype(ml_dtypes.bfloat16),
            "bq": np.ascontiguousarray((bq[sl] * sc).reshape(-1, 1)),
            "bk4": bk4,
            "bvb": np.ascontiguousarray(np.tile(bv[sl][None, :], (128, 1))),
        })

    nc = _build_bass()
    res = run_bass_kernel_spmd(
        nc, in_maps, core_ids=list(range(NCORES)),
        trace=os.environ.get("BASS_TRACE", "0") == "1",
    )
    LAST_RESULTS = res

    out = np.empty((B, HEADS, N, D), np.float32)
    for c in range(NCORES):
        b = c // CORES_PER_B
        h0 = HPC * (c % CORES_PER_B)
        oo = res.results[c]["o"]                  # [128, N] f32
        for hh in range(HPC):
            num = oo[hh * D:(hh + 1) * D, :]      # [32, N] unnormalized out^T
            den = oo[64 + 32 * hh, :]             # [N]
            out[b, h0 + hh] = (num / den[None, :]).T
    return out


# revision 34
# speedup vs baseline: 1.0097x; 1.0097x over previous
"""Adjacency-aware multi-head attention on 8 trn2 NeuronCores.

Math (per b, head k):
  Q = h[b] @ Wq[:, k] + bq[k]           [N, D]
  S[i, j] = (Q_i . K_j) / sqrt(D)
  P[j, i] = exp(S[i, j]) / sum_j exp(S[i, j])      (softmax over keys j)
  out[i, d] = sum_j P[j, i] * A[b, j, i] * V[j, d]

Sharding: 16 (b, head) pairs over 8 cores, 2 heads of the SAME b per core so
the A[b] stream is shared by both heads.

Device dataflow ([j, i] "transposed" layout so A needs no transpose):
  S^T[j-tile, i-chunk] on PE: K^T tile stationary (bf16), Q^T moving (bf16).
    K^T is packed so j-tile t lives on partition strip 32*(t%4) and Q^T is
    replicated to all 4 strips -> 4 j-tiles run CONCURRENTLY on the PE's
    four 32-row groups.
  exp on ACT: PSUM -> SBUF bf16 (groups of 3 PSUM banks per op)
  EA = E * A on DVE (bf16 tensor_tensor, 2x mode; A broadcast over heads)
  Phase 2 (lagged one chunk behind S so the in-order PE queue never stalls),
  4 concurrent col-group accumulation streams into one PSUM tile:
    cols  0-31: outT_h0 += V_h0[j]^T @ EA_h0     (M=32)
    cols 32-63: outT_h1 += V_h1[j]^T @ EA_h1     (M=32)
    col  64: denom_h0 += ones^T @ E_h0           (M=1)
    col  96: denom_h1 += ones^T @ E_h1           (M=1)
Device returns [128, N]: rows 0-31 outT_h0, 32-63 outT_h1, rows 64/96 the
softmax denominators.  Host does out = (outT / denom)^T plus the gather.
"""

import math
import os

import numpy as np
import ml_dtypes

B, N, IN_DIM = 2, 2048, 256
HEADS, D = 8, 32
NCORES = 8
HPC = 2              # heads per core
NJ = N // 128        # 16 j-tiles
NCH = 4              # i-chunks
CH = N // NCH        # 512
CORES_PER_B = NCORES // B

LAST_RESULTS = None  # BassKernelResults of the most recent kernel() call


def _build_bass():
    import concourse.bass as bass
    import concourse.mybir as mybir
    import concourse.tile as tile
    from concourse import bacc

    f32 = mybir.dt.float32
    bf16 = mybir.dt.bfloat16
    AF = mybir.ActivationFunctionType

    nc = bacc.Bacc("TRN2", target_bir_lowering=False, debug=False,
                   num_devices=NCORES)

    hT = nc.dram_tensor("hT", [IN_DIM, N], bf16, kind="ExternalInput").ap()
    Ab = nc.dram_tensor("Ab", [N, N], bf16, kind="ExternalInput").ap()
    wq = nc.dram_tensor("wq", [IN_DIM, HPC * D], bf16, kind="ExternalInput").ap()
    wk = nc.dram_tensor("wk", [IN_DIM, HPC * D], bf16, kind="ExternalInput").ap()
    wv = nc.dram_tensor("wv", [IN_DIM, HPC * D], bf16, kind="ExternalInput").ap()
    bq = nc.dram_tensor("bq", [HPC * D, 1], f32, kind="ExternalInput").ap()
    bk4 = nc.dram_tensor("bk4", [128, HPC], f32, kind="ExternalInput").ap()
    bvb = nc.dram_tensor("bvb", [128, HPC * D], f32, kind="ExternalInput").ap()
    o = nc.dram_tensor("o", [128, N], f32, kind="ExternalOutput").ap()

    SC = 1.0 / math.sqrt(D)

    with (
        tile.TileContext(nc) as tc,
        tc.tile_pool(name="const", bufs=1) as cpool,
        tc.tile_pool(name="ps", bufs=2, space="PSUM") as pspool,
        tc.tile_pool(name="pod", bufs=2, space="PSUM") as podpool,
        tc.tile_pool(name="apool", bufs=2) as apool,
        tc.tile_pool(name="epool", bufs=2) as epool,
        tc.tile_pool(name="eapool", bufs=2) as eapool,
        tc.tile_pool(name="opool", bufs=3) as opool,
    ):
        # ---- constants / inputs into SBUF
        hT_sb = cpool.tile([128, 2, N], bf16, tag="hT")
        nc.sync.dma_start(hT_sb, hT.rearrange("(s p) n -> p s n", p=128))
        w_sb = {}
        for name, ap in (("q", wq), ("k", wk), ("v", wv)):
            t = cpool.tile([128, 2, HPC * D], bf16, tag=f"w{name}")
            nc.sync.dma_start(t, ap.rearrange("(s p) m -> p s m", p=128))
            w_sb[name] = t
        bq_sb = cpool.tile([HPC * D, 1], f32, tag="bq")
        nc.sync.dma_start(bq_sb, bq)
        bk4_sb = cpool.tile([128, HPC], f32, tag="bk4")
        nc.sync.dma_start(bk4_sb, bk4)
        bvb_sb = cpool.tile([128, HPC * D], f32, tag="bvb")
        nc.sync.dma_start(bvb_sb, bvb)
        ones_sb = cpool.tile([128, 1], bf16, tag="ones")
        nc.vector.memset(ones_sb, 1.0)


        # Q^T replicated on all 4 partition strips; K^T packed so j-tile t
        # sits on strip 32*(t%4), column block t//4.
        qt4 = [cpool.tile([128, N], bf16, tag=f"qt4{h}", name=f"qt4{h}")
               for h in range(HPC)]
        kt4 = [cpool.tile([128, NJ // 4, 128], bf16, tag=f"kt4{h}",
                          name=f"kt4{h}") for h in range(HPC)]
        Vt = cpool.tile([128, NJ, HPC * D], bf16, tag="vt")

        def bcast_free(ap_col, n):
            return bass.AP(tensor=ap_col.tensor, offset=ap_col.offset,
                           ap=[ap_col.ap[0], [0, n]])

        # ---- K projection straight into the packed kt4 layout: for strip r
        #      the moving operand picks j-tiles {r, r+4, r+8, r+12}
        for h in range(HPC):
            ps = pspool.tile([128, 4 * 128], f32, tag="ps")
            for r in range(4):
                for s in range(2):
                    base = hT_sb[:, s, r * 128:(r + 1) * 128]
                    rhs = bass.AP(
                        tensor=base.tensor, offset=base.offset,
                        ap=[base.ap[0], [4 * 128, 4], [1, 128]],
                    )
                    nc.tensor.matmul(
                        ps[32 * r:32 * r + 32, :],
                        lhsT=w_sb["k"][:, s, h * D:(h + 1) * D],
                        rhs=rhs,
                        start=(s == 0), stop=(s == 1),
                        tile_position=(0, 32 * r),
                    )
            nc.vector.scalar_tensor_tensor(
                kt4[h].rearrange("p q jj -> p (q jj)"), ps, 1.0,
                bcast_free(bk4_sb[:, h:h + 1], 4 * 128),
                op0=mybir.AluOpType.mult, op1=mybir.AluOpType.add,
            )

        # ---- Q projection (+bias, scaled 1/sqrt(D)); each quarter is
        #      strip-replicated immediately so chunk 0's S matmuls (which
        #      only need quarter 0) can start as early as possible
        bq_bcast = bcast_free(bq_sb, CH)
        for quarter in range(4):
            sl = slice(quarter * CH, (quarter + 1) * CH)
            ps = pspool.tile([HPC * D, CH], f32, tag="ps")
            for s in range(2):
                nc.tensor.matmul(
                    ps, lhsT=w_sb["q"][:, s, :], rhs=hT_sb[:, s, sl],
                    start=(s == 0), stop=(s == 1),
                )
            nc.vector.scalar_tensor_tensor(
                qt4[0][0:D, sl], ps[0:D, :], SC, bq_bcast[0:D, :],
                op0=mybir.AluOpType.mult, op1=mybir.AluOpType.add,
            )
            nc.vector.scalar_tensor_tensor(
                qt4[1][D:2 * D, sl], ps[D:2 * D, :], SC, bq_bcast[D:2 * D, :],
                op0=mybir.AluOpType.mult, op1=mybir.AluOpType.add,
            )
            for r in range(1, 4):
                nc.sync.dma_start(qt4[0][32 * r:32 * r + 32, sl],
                                  qt4[0][0:D, sl])
            for r in (0, 2, 3):
                nc.sync.dma_start(qt4[1][32 * r:32 * r + 32, sl],
                                  qt4[1][D:2 * D, sl])

        # ---- projection V[j, d] for both heads (+bias via broadcast tile)
        for t in range(NJ):
            ps = pspool.tile([128, HPC * D], f32, tag="ps")
            for s in range(2):
                nc.tensor.matmul(
                    ps, lhsT=hT_sb[:, s, t * 128:(t + 1) * 128],
                    rhs=w_sb["v"][:, s, :],
                    start=(s == 0), stop=(s == 1),
                )
            nc.vector.tensor_add(Vt[:, t, :], ps, bvb_sb)

        # ---- main loop (phase2 lags one chunk so the in-order PE queue
        #      never stalls waiting on exp/A-mult of the same chunk)
        A3 = Ab.rearrange("(t p) i -> p t i", p=128)

        # e/ea column block for (head hh, j-tile t):
        def blk(hh, t):
            return (t // 4) * 8 + hh * 4 + (t % 4)

        def emit_phase2_tile(od, e_t, ea_t, t):
            first, last = (t == 0), (t == NJ - 1)
            sh0 = slice(blk(0, t) * CH, (blk(0, t) + 1) * CH)
            sh1 = slice(blk(1, t) * CH, (blk(1, t) + 1) * CH)
            nc.tensor.matmul(
                od[0:D, :], lhsT=Vt[:, t, 0:D], rhs=ea_t[:, sh0],
                start=first, stop=last, tile_position=(0, 0),
            )
            nc.tensor.matmul(
                od[D:2 * D, :], lhsT=Vt[:, t, D:2 * D], rhs=ea_t[:, sh1],
                start=first, stop=last, tile_position=(0, 32),
            )
            nc.tensor.matmul(
                od[64:65, :], lhsT=ones_sb, rhs=e_t[:, sh0],
                start=first, stop=last, tile_position=(0, 64),
            )
            nc.tensor.matmul(
                od[96:97, :], lhsT=ones_sb, rhs=e_t[:, sh1],
                start=first, stop=last, tile_position=(0, 96),
            )

        def emit_phase2(e_t, ea_t, ch):
            od = podpool.tile([128, CH], f32, tag="od")
            for t in range(NJ):
                emit_phase2_tile(od, e_t, ea_t, t)
            o_sb = opool.tile([128, CH], f32, tag="o")
            nc.vector.tensor_copy(o_sb, od)
            nc.sync.dma_start(o[:, ch * CH:(ch + 1) * CH], o_sb)

        GRP = 3              # S psum banks per exp op
        nblocks = NJ * HPC
        pending = None
        for ch in range(NCH):
            a_t = apool.tile([128, NJ * CH], bf16, tag="a")
            nc.sync.dma_start(
                a_t.rearrange("p (t i) -> p t i", i=CH),
                A3[:, :, ch * CH:(ch + 1) * CH],
            )
            e_t = epool.tile([128, nblocks * CH], bf16, tag="e")
            ea_t = eapool.tile([128, nblocks * CH], bf16, tag="ea")
            # S matmuls: block b = q*8 + hh*4 + r covers j-tile t = 4q + r;
            # runs of 4 same-head blocks occupy the 4 PE row-groups and run
            # concurrently.  exp drains GRP psum banks per op.  phase2 MMs
            # of the previous chunk are interleaved to fill exp-gated PE
            # stalls.
            ps = None
            gsz = 0
            for b in range(nblocks):
                q, hh, r = b // 8, (b // 4) % 2, b % 4
                g = b % GRP
                if g == 0:
                    gsz = min(GRP, nblocks - b)
                    ps = pspool.tile([128, GRP * CH], f32, tag="ps")
                nc.tensor.matmul(
                    ps[:, g * CH:(g + 1) * CH],
                    lhsT=kt4[hh][32 * r:32 * r + 32, q, :],
                    rhs=qt4[hh][32 * r:32 * r + 32, ch * CH:(ch + 1) * CH],
                    start=True, stop=True,
                    tile_position=(32 * r, 0),
                )
                if g == gsz - 1:
                    b0 = b - g
                    nc.scalar.activation(
                        e_t[:, b0 * CH:(b + 1) * CH], ps[:, :gsz * CH], AF.Exp)
            for t in range(NJ):
                b0 = blk(0, t)
                base = e_t[:, b0 * CH:(b0 + 1) * CH]
                e_pair = bass.AP(tensor=base.tensor, offset=base.offset,
                                 ap=[base.ap[0], [4 * CH, 2], [1, CH]])
                base2 = ea_t[:, b0 * CH:(b0 + 1) * CH]
                ea_pair = bass.AP(tensor=base2.tensor, offset=base2.offset,
                                  ap=[base2.ap[0], [4 * CH, 2], [1, CH]])
                a_sl = a_t[:, t * CH:(t + 1) * CH]
                a_bcast = bass.AP(tensor=a_sl.tensor, offset=a_sl.offset,
                                  ap=[a_sl.ap[0], [0, HPC], a_sl.ap[1]])
                nc.vector.tensor_mul(ea_pair, e_pair, a_bcast)
            if pending is not None:
                emit_phase2(*pending)
            pending = (e_t, ea_t, ch)
        emit_phase2(*pending)

    nc.finalize()
    return nc


def kernel(h, A, Wq, bq, Wk, bk, Wv, bv):
    global LAST_RESULTS
    from concourse.bass_utils import run_bass_kernel_spmd

    h = np.asarray(h, np.float32)
    A = np.asarray(A, np.float32)
    Wq = np.asarray(Wq, np.float32)
    Wk = np.asarray(Wk, np.float32)
    Wv = np.asarray(Wv, np.float32)
    bq = np.asarray(bq, np.float32)
    bk = np.asarray(bk, np.float32)
    bv = np.asarray(bv, np.float32)

    hT = np.ascontiguousarray(h.transpose(0, 2, 1)).astype(ml_dtypes.bfloat16)
    Ab = np.ascontiguousarray(A.astype(ml_dtypes.bfloat16))  # [B, N, N]
    sc = np.float32(1.0 / math.sqrt(D))

    in_maps = []
    for c in range(NCORES):
        b = c // CORES_PER_B
        h0 = HPC * (c % CORES_PER_B)
        sl = slice(h0 * D, (h0 + HPC) * D)
        bk2 = bk[sl].reshape(HPC, D)                    # [head, d]
        bk4 = np.empty((128, HPC), np.float32)
        for hh in range(HPC):
            bk4[:, hh] = np.tile(bk2[hh], 4)            # strip-replicated
        in_maps.append({
            "hT": hT[b],
            "Ab": Ab[b],
            "wq": np.ascontiguousarray(Wq[:, sl]).astype(ml_dtypes.bfloat16),
            "wk": np.ascontiguousarray(Wk[:, sl]).astype(ml_dtypes.bfloat16),
            "wv": np.ascontiguousarray(Wv[:, sl]).astype(ml_dtypes.bfloat16),
            "bq": np.ascontiguousarray((bq[sl] * sc).reshape(-1, 1)),
            "bk4": bk4,
            "bvb": np.ascontiguousarray(np.tile(bv[sl][None, :], (128, 1))),
        })

    nc = _build_bass()
    res = run_bass_kernel_spmd(
        nc, in_maps, core_ids=list(range(NCORES)),
        trace=os.environ.get("BASS_TRACE", "0") == "1",
    )
    LAST_RESULTS = res

    out = np.empty((B, HEADS, N, D), np.float32)
    for c in range(NCORES):
        b = c // CORES_PER_B
        h0 = HPC * (c % CORES_PER_B)
        oo = res.results[c]["o"]                  # [128, N] f32
        for hh in range(HPC):
            num = oo[hh * D:(hh + 1) * D, :]      # [32, N] unnormalized out^T
            den = oo[64 + 32 * hh, :]             # [N]
            out[b, h0 + hh] = (num / den[None, :]).T
    return out
